# revision 8
# baseline (speedup 1.0000x reference)
"""Depth-aware 3x3 conv (Depth-aware CNN) Trainium2 Bass kernel.

out(b,o,y,x) = sum_{c,kh,kw} W(o,c,kh,kw) * x(b,c,y+kh-1,x+kw-1)
              * exp(-ALPHA*|D(b,y,x) - D(b,y+kh-1,x+kw-1)|) + bias(o)

8 NeuronCores, data-parallel over batch B=8 (one image per core).

Device-side structure per core (v2):
  - The 9 taps pair up: sim_{-d}(p) = sim_{+d}(p-d), so only 4 "edge"
    similarity fields exist; the center tap has sim == 1.  Edge fields are
    host-precomputed and shipped per 16-row chunk as one contiguous
    [4, 17*132] block.
  - Per chunk: ONE GPSIMD partition_broadcast (bitcast to uint64: 8B/elem)
    replicates all 4 edge fields across the 128 Cin partitions at once.
  - DVE forms per edge two products straight from the padded input xp
    (no shifted copy; odd-element offsets keep the 2x DVE mode):
       zp  = shift(x) * e   (serves tap +d directly)
       phi = x        * e   (serves tap -d via shifted matmul reads)
    The 4th edge's zp runs on Pool (gpsimd tensor_tensor) to balance
    DVE (~63us) vs Pool (~61us) under the PE roofline (~62us).
  - TensorE: per 512-pixel tile, 9 bf16 matmuls accumulate in PSUM.  A few
    warmup matmuls on a zeroed scratch tile ramp the PE p-state during the
    startup DMAs.  ACT evacuates PSUM with a fused bias add; the last
    chunk evacuates in 512-px quarters so the tail DMA overlaps.
"""

import os
from contextlib import ExitStack

import ml_dtypes
import numpy as np

ALPHA = 8.3
B, C, H, W = 8, 128, 128, 128
HP, WP = H + 2, W + 2  # 130x130 padded image plane
N_CORES = 8
NPIX = H * W
EDGES = [(0, 1), (1, 0), (1, 1), (1, -1)]
G_ROWS = 16          # image rows per chunk (2048 pixels)
N_GROUPS = H // G_ROWS
MM_ROWS = 4          # image rows per matmul / psum tile (512 pixels)
MM_PER_G = G_ROWS // MM_ROWS
EW = 132             # edge-field row width (col c -> x col c-1)
ER = G_ROWS + 1      # edge-field rows per chunk
EBLK = ER * EW       # 2244 elems per edge field
N_WARM = 10          # PE p-state warmup matmuls

_cache = {}


def _build_bass():
    import concourse.bass as bass  # noqa: F401
    import concourse.tile as tile
    from concourse import bacc, library_config, mybir

    dt = mybir.dt
    u32 = dt.uint32
    nc = bacc.Bacc(
        "TRN2",
        target_bir_lowering=False,
        debug=False,
        enable_asserts=False,
        num_devices=N_CORES,
    )

    xpad_d = nc.dram_tensor("xpad", [C, HP * WP], dt.bfloat16, kind="ExternalInput").ap()
    simf_d = nc.dram_tensor("simf", [N_GROUPS, 2 * EBLK], dt.bfloat16, kind="ExternalInput").ap()
    simfb_d = nc.dram_tensor("simfb", [128, N_GROUPS * 2 * EBLK], dt.bfloat16, kind="ExternalInput").ap()
    wt_d = nc.dram_tensor("wt", [C, 9 * 128], dt.bfloat16, kind="ExternalInput").ap()
    bias_d = nc.dram_tensor("bias", [128, 1], dt.float32, kind="ExternalInput").ap()
    out_d = nc.dram_tensor("out", [128, NPIX], dt.float16, kind="ExternalOutput").ap()

    # weight slot order: 0 = center, then per edge (+d, -d)
    slot = {}
    s = 1
    for dy, dx in EDGES:
        slot[(dy, dx)] = s
        slot[(-dy, -dx)] = s + 1
        s += 2

    with tile.TileContext(nc) as tc, ExitStack() as ctx:
        nc.gpsimd.load_library(library_config.attn)

        big = ctx.enter_context(tc.tile_pool(name="big", bufs=1))
        sfp = ctx.enter_context(tc.tile_pool(name="sf", bufs=3))
        shp = ctx.enter_context(tc.tile_pool(name="sh", bufs=2))
        sbp = ctx.enter_context(tc.tile_pool(name="shb", bufs=3))
        php = ctx.enter_context(tc.tile_pool(name="phi", bufs=8))
        zpp = ctx.enter_context(tc.tile_pool(name="zpl", bufs=8))
        op_ = ctx.enter_context(tc.tile_pool(name="ost", bufs=4))
        pp = ctx.enter_context(
            tc.tile_pool(name="psum", bufs=4, space=bass.MemorySpace.PSUM)
        )

        xp = big.tile([C, HP * WP], dt.bfloat16)
        wt = big.tile([C, 9 * 128], dt.bfloat16)
        biast = big.tile([128, 1], dt.float32)
        scratch = big.tile([128, 512], dt.bfloat16)

        # zero scratch early so PE warmup matmuls have finite operands
        nc.gpsimd.memset(scratch[:], 0.0)

        # chunked input loads: chunk g's products need padded rows 16g..16g+17.
        segs = [(16 * k, 16 * (k + 1)) for k in range(8)] + [(128, HP)]

        def emit_seg(k):
            r0, r1 = segs[k]
            nc.sync.dma_start(
                xp[:, r0 * WP : r1 * WP], xpad_d[:, r0 * WP : r1 * WP]
            )

        # small transfers first so the first broadcast starts immediately
        sf_tiles = {}
        sb_tiles = {}

        def emit_sb(k):
            t = sbp.tile([128, 2 * EBLK], dt.bfloat16, tag="shb", name=f"sb{k}")
            nc.sync.dma_start(t[:], simfb_d[:, k * 2 * EBLK : (k + 1) * 2 * EBLK])
            sb_tiles[k] = t

        sf_tiles[0] = sfp.tile([1, 2 * EBLK], dt.bfloat16, tag="sf", name="sf0")
        nc.sync.dma_start(sf_tiles[0][:], simf_d[0:1, :])
        emit_sb(0)
        nc.sync.dma_start(biast[:], bias_d[:])
        nc.sync.dma_start(wt[:], wt_d[:])
        emit_seg(0)
        emit_seg(1)
        sf_tiles[1] = sfp.tile([1, 2 * EBLK], dt.bfloat16, tag="sf", name="sf1")
        nc.sync.dma_start(sf_tiles[1][:], simf_d[1:2, :])
        emit_sb(1)

        xpv = xp[:].rearrange("p (a b) -> p a b", b=WP)

        # PE p-state warmup: harmless matmuls on zeroed scratch while input
        # DMAs land.  They share the psum pool rotation; chunk 0's start=True
        # resets any accumulation state.
        warm = pp.tile([128, 2 * MM_ROWS * W], dt.float32, tag="ps", name="warm")
        for _ in range(N_WARM):
            nc.tensor.matmul(
                warm[:, 0:512], scratch[:, 0:128], scratch[:, 0:512],
                start=True, stop=True, skip_group_check=True,
            )

        for g in range(N_GROUPS):
            if g + 2 < len(segs):
                emit_seg(g + 2)
            if g + 2 < N_GROUPS:
                t = sfp.tile([1, 2 * EBLK], dt.bfloat16, tag="sf", name=f"sf{g+2}")
                nc.sync.dma_start(t[:], simf_d[g + 2 : g + 3, :])
                sf_tiles[g + 2] = t
                emit_sb(g + 2)
            y0 = g * G_ROWS

            sh23 = shp.tile([128, 2 * EBLK], dt.bfloat16, tag="sh")
            nc.gpsimd.partition_broadcast(
                sh23[:].bitcast(u32), sf_tiles[g][:].bitcast(u32)
            )
            sh01 = sb_tiles[g]

            phis = []
            zps = []
            for e, (dy, dx) in enumerate(EDGES):
                shsrc = sh01 if e < 2 else sh23
                shv = shsrc[:, (e % 2) * EBLK : (e % 2 + 1) * EBLK].rearrange(
                    "p (a b) -> p a b", b=EW
                )

                # zp[r, x] = x_grid(y0+r+dy+1, x+dx+1) * e(y0+r, x)
                zp = zpp.tile([128, G_ROWS, W], dt.bfloat16, tag="zp")
                src = xpv[:, 1 + y0 + dy : 1 + y0 + dy + G_ROWS, 1 + dx : 1 + dx + W]
                sim = shv[:, 1 : 1 + G_ROWS, 2 : 2 + W]
                if e == 3:
                    nc.gpsimd.tensor_tensor(zp[:], src, sim, op=mybir.AluOpType.mult)
                else:
                    nc.vector.tensor_tensor(zp[:], src, sim, op=mybir.AluOpType.mult)
                zps.append(zp)

                # phi[r, c] = x_grid(y0+rlo+r, c-1) * e(y0-1+rlo+r, c-2)
                # (tile row r holds field row rlo+r); -d tap reads cols
                # [c0, c0+W) with c0 = 2-dx
                rlo = 1 - dy
                c0 = 2 - dx
                # host zeroes sim col 1 for dx=+1 edges, so the product
                # itself writes the q_x=-1 zero column (no memset)
                phi = php.tile([128, G_ROWS, EW], dt.bfloat16, tag="phi")
                nc.vector.tensor_tensor(
                    phi[:, 0:G_ROWS, c0 : c0 + W],
                    xpv[:, y0 + rlo : y0 + rlo + G_ROWS, c0 - 1 : c0 - 1 + W],
                    shv[:, rlo : rlo + G_ROWS, c0 : c0 + W],
                    op=mybir.AluOpType.mult,
                )
                phis.append(phi)

            pstiles = [
                pp.tile([128, 2 * MM_ROWS * W], dt.float32, tag="ps", name=f"ps{g}_{j}")
                for j in range(MM_PER_G // 2)
            ]
            psums = [
                pstiles[j // 2][:, (j % 2) * MM_ROWS * W : (j % 2 + 1) * MM_ROWS * W]
                for j in range(MM_PER_G)
            ]
            for tt in range(9):
                if tt == 0:
                    pass
                else:
                    e = (tt - 1) // 2
                    dy, dx = EDGES[e]
                    plus = (tt - 1) % 2 == 0
                    wslot = slot[(dy, dx)] if plus else slot[(-dy, -dx)]
                for j in range(MM_PER_G):
                    if tt == 0:
                        rhs = xpv[:, 1 + y0 + j * MM_ROWS : 1 + y0 + (j + 1) * MM_ROWS, 1 : 1 + W]
                        ws = 0
                    elif plus:
                        rhs = zps[e][:, j * MM_ROWS : (j + 1) * MM_ROWS, :]
                        ws = wslot
                    else:
                        c0 = 2 - dx
                        rhs = phis[e][:, j * MM_ROWS : j * MM_ROWS + MM_ROWS, c0 : c0 + W]
                        ws = wslot
                    nc.tensor.matmul(
                        psums[j],
                        wt[:, ws * 128 : (ws + 1) * 128],
                        rhs,
                        start=(tt == 0),
                        stop=(tt == 8),
                        skip_group_check=True,
                    )

            if g < N_GROUPS - 1:
                for j in range(MM_PER_G // 2):
                    ost = op_.tile([128, 2 * MM_ROWS * W], dt.float16, tag="ost")
                    nc.scalar.activation(
                        ost[:],
                        pstiles[j][:],
                        mybir.ActivationFunctionType.Identity,
                        bias=biast[:, 0:1],
                    )
                    c0 = (y0 + 2 * j * MM_ROWS) * W
                    nc.scalar.dma_start(out_d[:, c0 : c0 + 2 * MM_ROWS * W], ost[:])
            else:
                # last chunk: quarter-granularity so the tail DMA overlaps
                for j in range(MM_PER_G):
                    ost = op_.tile([128, MM_ROWS * W], dt.float16, tag="ost4")
                    nc.scalar.activation(
                        ost[:],
                        psums[j],
                        mybir.ActivationFunctionType.Identity,
                        bias=biast[:, 0:1],
                    )
                    c0 = (y0 + j * MM_ROWS) * W
                    nc.scalar.dma_start(out_d[:, c0 : c0 + MM_ROWS * W], ost[:])

    nc.compile()
    return nc


def _get_nc():
    if "nc" not in _cache:
        _cache["nc"] = _build_bass()
    return _cache["nc"]


def _host_prep(input, depth, weight, bias):
    bf16 = ml_dtypes.bfloat16

    xpad = np.zeros((B, C, HP, WP), dtype=bf16)
    xpad[:, :, 1 : 1 + H, 1 : 1 + W] = input.astype(bf16)
    xpad = xpad.reshape(B, C, HP * WP)

    # edge similarity fields on a 132x132 grid: EF[b, qy+2, qx+2] =
    # exp(-a*|D(qy,qx) - D(qy+dy,qx+dx)|), D zero-padded.
    dext = np.zeros((B, H + 6, W + 6), dtype=np.float32)
    dext[:, 3 : 3 + H, 3 : 3 + W] = depth[:, 0, :, :]
    simf = np.empty((B, N_GROUPS, 4, ER, EW), dtype=bf16)
    for e, (dy, dx) in enumerate(EDGES):
        a = dext[:, 1 : 1 + 132, 1 : 1 + 132]
        bsh = dext[:, 1 + dy : 1 + dy + 132, 1 + dx : 1 + dx + 132]
        ef = np.exp(-ALPHA * np.abs(a - bsh)).astype(bf16)  # [B, 132, 132]
        if dx == 1:
            ef[:, :, 1] = 0  # q_x=-1 zero-pad column, written by the product
        for g in range(N_GROUPS):
            simf[:, g, e, :, :] = ef[:, 16 * g + 1 : 16 * g + 1 + ER, 0:EW]
    # edges 0,1 pre-broadcast across the 128 partitions (shipped via DMA);
    # edges 2,3 stay flat (device Pool broadcast)
    simfb = np.ascontiguousarray(
        np.broadcast_to(
            simf[:, None, :, 0:2].reshape(B, 1, N_GROUPS * 2 * EBLK),
            (B, 128, N_GROUPS * 2 * EBLK),
        )
    )
    simf = np.ascontiguousarray(simf[:, :, 2:4].reshape(B, N_GROUPS, 2 * EBLK))

    wt = np.empty((C, 9 * 128), dtype=bf16)
    wtr = weight.astype(np.float32).transpose(1, 2, 3, 0)  # [c, kh, kw, o]
    wt[:, 0:128] = wtr[:, 1, 1, :].astype(bf16)
    s = 1
    for dy, dx in EDGES:
        wt[:, s * 128 : (s + 1) * 128] = wtr[:, dy + 1, dx + 1, :].astype(bf16)
        wt[:, (s + 1) * 128 : (s + 2) * 128] = wtr[:, 1 - dy, 1 - dx, :].astype(bf16)
        s += 2

    bias2 = np.ascontiguousarray(bias.astype(np.float32).reshape(128, 1))
    return xpad, simf, simfb, wt, bias2


def kernel(input, depth, weight, bias):
    from concourse.bass_utils import run_bass_kernel_spmd

    nc = _get_nc()
    xpad, simf, simfb, wt, bias2 = _host_prep(input, depth, weight, bias)

    in_maps = []
    for b in range(B):
        in_maps.append(
            {
                "xpad": np.ascontiguousarray(xpad[b]),
                "simf": np.ascontiguousarray(simf[b]),
                "simfb": simfb[b],
                "wt": wt,
                "bias": bias2,
            }
        )

    trace = os.environ.get("KERNEL_TRACE", "0") == "1"
    res = run_bass_kernel_spmd(
        nc, in_maps, core_ids=list(range(N_CORES)), trace=trace
    )
    if trace:
        _cache["last_results"] = res

    out = np.stack(
        [res.results[b]["out"].astype(np.float32).reshape(128, H, W) for b in range(B)]
    )
    return out


# revision 9
# speedup vs baseline: 1.0191x; 1.0191x over previous
"""Depth-aware 3x3 conv (Depth-aware CNN) Trainium2 Bass kernel.

out(b,o,y,x) = sum_{c,kh,kw} W(o,c,kh,kw) * x(b,c,y+kh-1,x+kw-1)
              * exp(-ALPHA*|D(b,y,x) - D(b,y+kh-1,x+kw-1)|) + bias(o)

8 NeuronCores, data-parallel over batch B=8 (one image per core).

Device-side structure per core (v2):
  - The 9 taps pair up: sim_{-d}(p) = sim_{+d}(p-d), so only 4 "edge"
    similarity fields exist; the center tap has sim == 1.  Edge fields are
    host-precomputed and shipped per 16-row chunk as one contiguous
    [4, 17*132] block.
  - Per chunk: ONE GPSIMD partition_broadcast (bitcast to uint64: 8B/elem)
    replicates all 4 edge fields across the 128 Cin partitions at once.
  - DVE forms per edge two products straight from the padded input xp
    (no shifted copy; odd-element offsets keep the 2x DVE mode):
       zp  = shift(x) * e   (serves tap +d directly)
       phi = x        * e   (serves tap -d via shifted matmul reads)
    The 4th edge's zp runs on Pool (gpsimd tensor_tensor) to balance
    DVE (~63us) vs Pool (~61us) under the PE roofline (~62us).
  - TensorE: per 512-pixel tile, 9 bf16 matmuls accumulate in PSUM.  A few
    warmup matmuls on a zeroed scratch tile ramp the PE p-state during the
    startup DMAs.  ACT evacuates PSUM with a fused bias add; the last
    chunk evacuates in 512-px quarters so the tail DMA overlaps.
"""

import os
from contextlib import ExitStack

import ml_dtypes
import numpy as np

ALPHA = 8.3
B, C, H, W = 8, 128, 128, 128
HP, WP = H + 2, W + 2  # 130x130 padded image plane
N_CORES = 8
NPIX = H * W
EDGES = [(0, 1), (1, 0), (1, 1), (1, -1)]
G_ROWS = 16          # image rows per chunk (2048 pixels)
N_GROUPS = H // G_ROWS
MM_ROWS = 4          # image rows per matmul / psum tile (512 pixels)
MM_PER_G = G_ROWS // MM_ROWS
EW = 132             # edge-field row width (col c -> x col c-1)
ER = G_ROWS + 1      # edge-field rows per chunk
EBLK = ER * EW       # 2244 elems per edge field
N_WARM = 30          # PE p-state warmup matmuls

_cache = {}


def _build_bass():
    import concourse.bass as bass  # noqa: F401
    import concourse.tile as tile
    from concourse import bacc, library_config, mybir

    dt = mybir.dt
    u32 = dt.uint32
    nc = bacc.Bacc(
        "TRN2",
        target_bir_lowering=False,
        debug=False,
        enable_asserts=False,
        num_devices=N_CORES,
    )

    xpad_d = nc.dram_tensor("xpad", [C, HP * WP], dt.bfloat16, kind="ExternalInput").ap()
    simf_d = nc.dram_tensor("simf", [N_GROUPS, 2 * EBLK], dt.bfloat16, kind="ExternalInput").ap()
    simfb_d = nc.dram_tensor("simfb", [128, N_GROUPS * 2 * EBLK], dt.bfloat16, kind="ExternalInput").ap()
    wt_d = nc.dram_tensor("wt", [C, 9 * 128], dt.bfloat16, kind="ExternalInput").ap()
    bias_d = nc.dram_tensor("bias", [128, 1], dt.float32, kind="ExternalInput").ap()
    out_d = nc.dram_tensor("out", [128, NPIX], dt.float16, kind="ExternalOutput").ap()

    # weight slot order: 0 = center, then per edge (+d, -d)
    slot = {}
    s = 1
    for dy, dx in EDGES:
        slot[(dy, dx)] = s
        slot[(-dy, -dx)] = s + 1
        s += 2

    with tile.TileContext(nc) as tc, ExitStack() as ctx:
        nc.gpsimd.load_library(library_config.attn)

        big = ctx.enter_context(tc.tile_pool(name="big", bufs=1))
        sfp = ctx.enter_context(tc.tile_pool(name="sf", bufs=3))
        shp = ctx.enter_context(tc.tile_pool(name="sh", bufs=2))
        sbp = ctx.enter_context(tc.tile_pool(name="shb", bufs=3))
        php = ctx.enter_context(tc.tile_pool(name="phi", bufs=8))
        zpp = ctx.enter_context(tc.tile_pool(name="zpl", bufs=8))
        op_ = ctx.enter_context(tc.tile_pool(name="ost", bufs=4))
        pp = ctx.enter_context(
            tc.tile_pool(name="psum", bufs=4, space=bass.MemorySpace.PSUM)
        )

        xp = big.tile([C, HP * WP], dt.bfloat16)
        wt = big.tile([C, 9 * 128], dt.bfloat16)
        biast = big.tile([128, 1], dt.float32)
        scratch = big.tile([128, 512], dt.bfloat16)

        # zero scratch early so PE warmup matmuls have finite operands
        nc.gpsimd.memset(scratch[:], 0.0)

        # chunked input loads: chunk g's products need padded rows 16g..16g+17.
        segs = [(16 * k, 16 * (k + 1)) for k in range(8)] + [(128, HP)]

        def emit_seg(k):
            r0, r1 = segs[k]
            nc.sync.dma_start(
                xp[:, r0 * WP : r1 * WP], xpad_d[:, r0 * WP : r1 * WP]
            )

        # small transfers first so the first broadcast starts immediately
        sf_tiles = {}
        sb_tiles = {}

        def emit_sb(k):
            t = sbp.tile([128, 2 * EBLK], dt.bfloat16, tag="shb", name=f"sb{k}")
            nc.sync.dma_start(t[:], simfb_d[:, k * 2 * EBLK : (k + 1) * 2 * EBLK])
            sb_tiles[k] = t

        sf_tiles[0] = sfp.tile([1, 2 * EBLK], dt.bfloat16, tag="sf", name="sf0")
        nc.sync.dma_start(sf_tiles[0][:], simf_d[0:1, :])
        # chunk-0 critical path first: edge-0 sim block + input rows
        t0 = sbp.tile([128, 2 * EBLK], dt.bfloat16, tag="shb", name="sb0")
        nc.sync.dma_start(t0[:, 0:EBLK], simfb_d[:, 0:EBLK])
        sb_tiles[0] = t0
        emit_seg(0)
        emit_seg(1)
        nc.sync.dma_start(t0[:, EBLK : 2 * EBLK], simfb_d[:, EBLK : 2 * EBLK])
        nc.sync.dma_start(wt[:], wt_d[:])
        nc.sync.dma_start(biast[:], bias_d[:])
        sf_tiles[1] = sfp.tile([1, 2 * EBLK], dt.bfloat16, tag="sf", name="sf1")
        nc.sync.dma_start(sf_tiles[1][:], simf_d[1:2, :])
        emit_sb(1)

        xpv = xp[:].rearrange("p (a b) -> p a b", b=WP)

        # PE p-state warmup: harmless matmuls on zeroed scratch while input
        # DMAs land.  They share the psum pool rotation; chunk 0's start=True
        # resets any accumulation state.
        warm = pp.tile([128, 2 * MM_ROWS * W], dt.float32, tag="ps", name="warm")
        for _ in range(N_WARM):
            nc.tensor.matmul(
                warm[:, 0:512], scratch[:, 0:128], scratch[:, 0:512],
                start=True, stop=True, skip_group_check=True,
            )

        for g in range(N_GROUPS):
            if g + 2 < len(segs):
                emit_seg(g + 2)
            if g + 2 < N_GROUPS:
                t = sfp.tile([1, 2 * EBLK], dt.bfloat16, tag="sf", name=f"sf{g+2}")
                nc.sync.dma_start(t[:], simf_d[g + 2 : g + 3, :])
                sf_tiles[g + 2] = t
                emit_sb(g + 2)
            y0 = g * G_ROWS

            sh23 = shp.tile([128, 2 * EBLK], dt.bfloat16, tag="sh")
            nc.gpsimd.partition_broadcast(
                sh23[:].bitcast(u32), sf_tiles[g][:].bitcast(u32)
            )
            sh01 = sb_tiles[g]

            phis = []
            zps = []
            for e, (dy, dx) in enumerate(EDGES):
                shsrc = sh01 if e < 2 else sh23
                shv = shsrc[:, (e % 2) * EBLK : (e % 2 + 1) * EBLK].rearrange(
                    "p (a b) -> p a b", b=EW
                )

                # zp[r, x] = x_grid(y0+r+dy+1, x+dx+1) * e(y0+r, x)
                zp = zpp.tile([128, G_ROWS, W], dt.bfloat16, tag="zp")
                src = xpv[:, 1 + y0 + dy : 1 + y0 + dy + G_ROWS, 1 + dx : 1 + dx + W]
                sim = shv[:, 1 : 1 + G_ROWS, 2 : 2 + W]
                if e == 3:
                    nc.gpsimd.tensor_tensor(zp[:], src, sim, op=mybir.AluOpType.mult)
                else:
                    nc.vector.tensor_tensor(zp[:], src, sim, op=mybir.AluOpType.mult)
                zps.append(zp)

                # phi[r, c] = x_grid(y0+rlo+r, c-1) * e(y0-1+rlo+r, c-2)
                # (tile row r holds field row rlo+r); -d tap reads cols
                # [c0, c0+W) with c0 = 2-dx
                rlo = 1 - dy
                c0 = 2 - dx
                # host zeroes sim col 1 for dx=+1 edges, so the product
                # itself writes the q_x=-1 zero column (no memset)
                phi = php.tile([128, G_ROWS, EW], dt.bfloat16, tag="phi")
                nc.vector.tensor_tensor(
                    phi[:, 0:G_ROWS, c0 : c0 + W],
                    xpv[:, y0 + rlo : y0 + rlo + G_ROWS, c0 - 1 : c0 - 1 + W],
                    shv[:, rlo : rlo + G_ROWS, c0 : c0 + W],
                    op=mybir.AluOpType.mult,
                )
                phis.append(phi)

            pstiles = [
                pp.tile([128, 2 * MM_ROWS * W], dt.float32, tag="ps", name=f"ps{g}_{j}")
                for j in range(MM_PER_G // 2)
            ]
            psums = [
                pstiles[j // 2][:, (j % 2) * MM_ROWS * W : (j % 2 + 1) * MM_ROWS * W]
                for j in range(MM_PER_G)
            ]
            for tt in range(9):
                if tt == 0:
                    pass
                else:
                    e = (tt - 1) // 2
                    dy, dx = EDGES[e]
                    plus = (tt - 1) % 2 == 0
                    wslot = slot[(dy, dx)] if plus else slot[(-dy, -dx)]
                for j in range(MM_PER_G):
                    if tt == 0:
                        rhs = xpv[:, 1 + y0 + j * MM_ROWS : 1 + y0 + (j + 1) * MM_ROWS, 1 : 1 + W]
                        ws = 0
                    elif plus:
                        rhs = zps[e][:, j * MM_ROWS : (j + 1) * MM_ROWS, :]
                        ws = wslot
                    else:
                        c0 = 2 - dx
                        rhs = phis[e][:, j * MM_ROWS : j * MM_ROWS + MM_ROWS, c0 : c0 + W]
                        ws = wslot
                    nc.tensor.matmul(
                        psums[j],
                        wt[:, ws * 128 : (ws + 1) * 128],
                        rhs,
                        start=(tt == 0),
                        stop=(tt == 8),
                        skip_group_check=True,
                    )

            if g < N_GROUPS - 1:
                for j in range(MM_PER_G // 2):
                    ost = op_.tile([128, 2 * MM_ROWS * W], dt.float16, tag="ost")
                    nc.scalar.activation(
                        ost[:],
                        pstiles[j][:],
                        mybir.ActivationFunctionType.Identity,
                        bias=biast[:, 0:1],
                    )
                    c0 = (y0 + 2 * j * MM_ROWS) * W
                    nc.scalar.dma_start(out_d[:, c0 : c0 + 2 * MM_ROWS * W], ost[:])
            else:
                # last chunk: quarter-granularity so the tail DMA overlaps
                for j in range(MM_PER_G):
                    ost = op_.tile([128, MM_ROWS * W], dt.float16, tag="ost4")
                    nc.scalar.activation(
                        ost[:],
                        psums[j],
                        mybir.ActivationFunctionType.Identity,
                        bias=biast[:, 0:1],
                    )
                    c0 = (y0 + j * MM_ROWS) * W
                    nc.scalar.dma_start(out_d[:, c0 : c0 + MM_ROWS * W], ost[:])

    nc.compile()
    return nc


def _get_nc():
    if "nc" not in _cache:
        _cache["nc"] = _build_bass()
    return _cache["nc"]


def _host_prep(input, depth, weight, bias):
    bf16 = ml_dtypes.bfloat16

    xpad = np.zeros((B, C, HP, WP), dtype=bf16)
    xpad[:, :, 1 : 1 + H, 1 : 1 + W] = input.astype(bf16)
    xpad = xpad.reshape(B, C, HP * WP)

    # edge similarity fields on a 132x132 grid: EF[b, qy+2, qx+2] =
    # exp(-a*|D(qy,qx) - D(qy+dy,qx+dx)|), D zero-padded.
    dext = np.zeros((B, H + 6, W + 6), dtype=np.float32)
    dext[:, 3 : 3 + H, 3 : 3 + W] = depth[:, 0, :, :]
    simf = np.empty((B, N_GROUPS, 4, ER, EW), dtype=bf16)
    for e, (dy, dx) in enumerate(EDGES):
        a = dext[:, 1 : 1 + 132, 1 : 1 + 132]
        bsh = dext[:, 1 + dy : 1 + dy + 132, 1 + dx : 1 + dx + 132]
        ef = np.exp(-ALPHA * np.abs(a - bsh)).astype(bf16)  # [B, 132, 132]
        if dx == 1:
            ef[:, :, 1] = 0  # q_x=-1 zero-pad column, written by the product
        for g in range(N_GROUPS):
            simf[:, g, e, :, :] = ef[:, 16 * g + 1 : 16 * g + 1 + ER, 0:EW]
    # edges 0,1 pre-broadcast across the 128 partitions (shipped via DMA);
    # edges 2,3 stay flat (device Pool broadcast)
    simfb = np.ascontiguousarray(
        np.broadcast_to(
            simf[:, None, :, 0:2].reshape(B, 1, N_GROUPS * 2 * EBLK),
            (B, 128, N_GROUPS * 2 * EBLK),
        )
    )
    simf = np.ascontiguousarray(simf[:, :, 2:4].reshape(B, N_GROUPS, 2 * EBLK))

    wt = np.empty((C, 9 * 128), dtype=bf16)
    wtr = weight.astype(np.float32).transpose(1, 2, 3, 0)  # [c, kh, kw, o]
    wt[:, 0:128] = wtr[:, 1, 1, :].astype(bf16)
    s = 1
    for dy, dx in EDGES:
        wt[:, s * 128 : (s + 1) * 128] = wtr[:, dy + 1, dx + 1, :].astype(bf16)
        wt[:, (s + 1) * 128 : (s + 2) * 128] = wtr[:, 1 - dy, 1 - dx, :].astype(bf16)
        s += 2

    bias2 = np.ascontiguousarray(bias.astype(np.float32).reshape(128, 1))
    return xpad, simf, simfb, wt, bias2


def kernel(input, depth, weight, bias):
    from concourse.bass_utils import run_bass_kernel_spmd

    nc = _get_nc()
    xpad, simf, simfb, wt, bias2 = _host_prep(input, depth, weight, bias)

    in_maps = []
    for b in range(B):
        in_maps.append(
            {
                "xpad": np.ascontiguousarray(xpad[b]),
                "simf": np.ascontiguousarray(simf[b]),
                "simfb": simfb[b],
                "wt": wt,
                "bias": bias2,
            }
        )

    trace = os.environ.get("KERNEL_TRACE", "0") == "1"
    res = run_bass_kernel_spmd(
        nc, in_maps, core_ids=list(range(N_CORES)), trace=trace
    )
    if trace:
        _cache["last_results"] = res

    out = np.stack(
        [res.results[b]["out"].astype(np.float32).reshape(128, H, W) for b in range(B)]
    )
    return out


# revision 10
# speedup vs baseline: 1.0399x; 1.0204x over previous
"""Depth-aware 3x3 conv (Depth-aware CNN) Trainium2 Bass kernel.

out(b,o,y,x) = sum_{c,kh,kw} W(o,c,kh,kw) * x(b,c,y+kh-1,x+kw-1)
              * exp(-ALPHA*|D(b,y,x) - D(b,y+kh-1,x+kw-1)|) + bias(o)

8 NeuronCores, data-parallel over batch B=8 (one image per core).

Device-side structure per core (v2):
  - The 9 taps pair up: sim_{-d}(p) = sim_{+d}(p-d), so only 4 "edge"
    similarity fields exist; the center tap has sim == 1.  Edge fields are
    host-precomputed and shipped per 16-row chunk as one contiguous
    [4, 17*132] block.
  - Per chunk: ONE GPSIMD partition_broadcast (bitcast to uint64: 8B/elem)
    replicates all 4 edge fields across the 128 Cin partitions at once.
  - DVE forms per edge two products straight from the padded input xp
    (no shifted copy; odd-element offsets keep the 2x DVE mode):
       zp  = shift(x) * e   (serves tap +d directly)
       phi = x        * e   (serves tap -d via shifted matmul reads)
    The 4th edge's zp runs on Pool (gpsimd tensor_tensor) to balance
    DVE (~63us) vs Pool (~61us) under the PE roofline (~62us).
  - TensorE: per 512-pixel tile, 9 bf16 matmuls accumulate in PSUM.  A few
    warmup matmuls on a zeroed scratch tile ramp the PE p-state during the
    startup DMAs.  ACT evacuates PSUM with a fused bias add; the last
    chunk evacuates in 512-px quarters so the tail DMA overlaps.
"""

import os
from contextlib import ExitStack

import ml_dtypes
import numpy as np

ALPHA = 8.3
B, C, H, W = 8, 128, 128, 128
HP, WP = H + 2, W + 2  # 130x130 padded image plane
N_CORES = 8
NPIX = H * W
EDGES = [(0, 1), (1, 0), (1, 1), (1, -1)]
G_ROWS = 16          # image rows per chunk (2048 pixels)
N_GROUPS = H // G_ROWS
MM_ROWS = 4          # image rows per matmul / psum tile (512 pixels)
MM_PER_G = G_ROWS // MM_ROWS
EW = 132             # edge-field row width (col c -> x col c-1)
ER = G_ROWS + 1      # edge-field rows per chunk
EBLK = ER * EW       # 2244 elems per edge field
N_WARM = 30          # PE p-state warmup matmuls

_cache = {}


def _build_bass():
    import concourse.bass as bass  # noqa: F401
    import concourse.tile as tile
    from concourse import bacc, library_config, mybir

    dt = mybir.dt
    u32 = dt.uint32
    nc = bacc.Bacc(
        "TRN2",
        target_bir_lowering=False,
        debug=False,
        enable_asserts=False,
        num_devices=N_CORES,
    )

    xpad_d = nc.dram_tensor("xpad", [C, HP * WP], dt.bfloat16, kind="ExternalInput").ap()
    simf_d = nc.dram_tensor("simf", [N_GROUPS, 2 * EBLK], dt.bfloat16, kind="ExternalInput").ap()
    simfb_d = nc.dram_tensor("simfb", [128, N_GROUPS * 2 * EBLK], dt.bfloat16, kind="ExternalInput").ap()
    wt_d = nc.dram_tensor("wt", [C, 9 * 128], dt.bfloat16, kind="ExternalInput").ap()
    bias_d = nc.dram_tensor("bias", [128, 1], dt.float32, kind="ExternalInput").ap()
    out_d = nc.dram_tensor("out", [128, NPIX], dt.float16, kind="ExternalOutput").ap()

    # weight slot order: 0 = center, then per edge (+d, -d)
    slot = {}
    s = 1
    for dy, dx in EDGES:
        slot[(dy, dx)] = s
        slot[(-dy, -dx)] = s + 1
        s += 2

    with tile.TileContext(nc) as tc, ExitStack() as ctx:
        nc.gpsimd.load_library(library_config.attn)

        big = ctx.enter_context(tc.tile_pool(name="big", bufs=1))
        sfp = ctx.enter_context(tc.tile_pool(name="sf", bufs=3))
        shp = ctx.enter_context(tc.tile_pool(name="sh", bufs=2))
        sbp = ctx.enter_context(tc.tile_pool(name="shb", bufs=3))
        php = ctx.enter_context(tc.tile_pool(name="phi", bufs=8))
        zpp = ctx.enter_context(tc.tile_pool(name="zpl", bufs=8))
        op_ = ctx.enter_context(tc.tile_pool(name="ost", bufs=4))
        pp = ctx.enter_context(
            tc.tile_pool(name="psum", bufs=4, space=bass.MemorySpace.PSUM)
        )

        xp = big.tile([C, HP * WP], dt.bfloat16)
        wt = big.tile([C, 9 * 128], dt.bfloat16)
        biast = big.tile([128, 1], dt.float32)
        scratch = big.tile([128, 512], dt.bfloat16)

        # zero scratch early so PE warmup matmuls have finite operands
        nc.gpsimd.memset(scratch[:], 0.0)

        # chunked input loads: chunk g's products need padded rows 16g..16g+17.
        segs = [(16 * k, 16 * (k + 1)) for k in range(8)] + [(128, HP)]

        def emit_seg(k):
            r0, r1 = segs[k]
            nc.sync.dma_start(
                xp[:, r0 * WP : r1 * WP], xpad_d[:, r0 * WP : r1 * WP]
            )

        # small transfers first so the first broadcast starts immediately
        sf_tiles = {}
        sb_tiles = {}

        def emit_sb(k):
            t = sbp.tile([128, 2 * EBLK], dt.bfloat16, tag="shb", name=f"sb{k}")
            nc.sync.dma_start(t[:], simfb_d[:, k * 2 * EBLK : (k + 1) * 2 * EBLK])
            sb_tiles[k] = t

        sf_tiles[0] = sfp.tile([1, 2 * EBLK], dt.bfloat16, tag="sf", name="sf0")
        nc.sync.dma_start(sf_tiles[0][:], simf_d[0:1, :])
        # chunk-0 critical path first: edge-0 sim block + input rows
        t0 = sbp.tile([128, 2 * EBLK], dt.bfloat16, tag="shb", name="sb0")
        nc.sync.dma_start(t0[:, 0:EBLK], simfb_d[:, 0:EBLK])
        sb_tiles[0] = t0
        emit_seg(0)
        emit_seg(1)
        nc.sync.dma_start(t0[:, EBLK : 2 * EBLK], simfb_d[:, EBLK : 2 * EBLK])
        nc.sync.dma_start(wt[:], wt_d[:])
        nc.sync.dma_start(biast[:], bias_d[:])
        sf_tiles[1] = sfp.tile([1, 2 * EBLK], dt.bfloat16, tag="sf", name="sf1")
        nc.sync.dma_start(sf_tiles[1][:], simf_d[1:2, :])
        emit_sb(1)

        xpv = xp[:].rearrange("p (a b) -> p a b", b=WP)

        # PE p-state warmup: harmless matmuls on zeroed scratch while input
        # DMAs land.  They share the psum pool rotation; chunk 0's start=True
        # resets any accumulation state.
        warm = pp.tile([128, 2 * MM_ROWS * W], dt.float32, tag="ps", name="warm")
        for _ in range(N_WARM):
            nc.tensor.matmul(
                warm[:, 0:512], scratch[:, 0:128], scratch[:, 0:512],
                start=True, stop=True, skip_group_check=True,
            )

        for g in range(N_GROUPS):
            if g + 2 < len(segs):
                emit_seg(g + 2)
            if g + 2 < N_GROUPS:
                t = sfp.tile([1, 2 * EBLK], dt.bfloat16, tag="sf", name=f"sf{g+2}")
                nc.sync.dma_start(t[:], simf_d[g + 2 : g + 3, :])
                sf_tiles[g + 2] = t
                emit_sb(g + 2)
            y0 = g * G_ROWS

            sh23 = shp.tile([128, 2 * EBLK], dt.bfloat16, tag="sh")
            nc.gpsimd.partition_broadcast(
                sh23[:].bitcast(u32), sf_tiles[g][:].bitcast(u32)
            )
            sh01 = sb_tiles[g]

            phis = []
            zps = []
            for e, (dy, dx) in enumerate(EDGES):
                shsrc = sh01 if e < 2 else sh23
                shv = shsrc[:, (e % 2) * EBLK : (e % 2 + 1) * EBLK].rearrange(
                    "p (a b) -> p a b", b=EW
                )

                # zp[r, x] = x_grid(y0+r+dy+1, x+dx+1) * e(y0+r, x)
                zp = zpp.tile([128, G_ROWS, W], dt.bfloat16, tag="zp")
                src = xpv[:, 1 + y0 + dy : 1 + y0 + dy + G_ROWS, 1 + dx : 1 + dx + W]
                sim = shv[:, 1 : 1 + G_ROWS, 2 : 2 + W]
                if e == 3:
                    nc.gpsimd.tensor_tensor(zp[:], src, sim, op=mybir.AluOpType.mult)
                else:
                    nc.vector.tensor_tensor(zp[:], src, sim, op=mybir.AluOpType.mult)
                zps.append(zp)

                # phi[r, c] = x_grid(y0+rlo+r, c-1) * e(y0-1+rlo+r, c-2)
                # (tile row r holds field row rlo+r); -d tap reads cols
                # [c0, c0+W) with c0 = 2-dx
                rlo = 1 - dy
                c0 = 2 - dx
                # host zeroes sim col 1 for dx=+1 edges, so the product
                # itself writes the q_x=-1 zero column (no memset)
                phi = php.tile([128, G_ROWS, EW], dt.bfloat16, tag="phi")
                nc.vector.tensor_tensor(
                    phi[:, 0:G_ROWS, c0 : c0 + W],
                    xpv[:, y0 + rlo : y0 + rlo + G_ROWS, c0 - 1 : c0 - 1 + W],
                    shv[:, rlo : rlo + G_ROWS, c0 : c0 + W],
                    op=mybir.AluOpType.mult,
                )
                phis.append(phi)

            pstiles = [
                pp.tile([128, 2 * MM_ROWS * W], dt.float32, tag="ps", name=f"ps{g}_{j}")
                for j in range(MM_PER_G // 2)
            ]
            psums = [
                pstiles[j // 2][:, (j % 2) * MM_ROWS * W : (j % 2 + 1) * MM_ROWS * W]
                for j in range(MM_PER_G)
            ]
            for tt in range(9):
                if tt == 0:
                    pass
                else:
                    e = (tt - 1) // 2
                    dy, dx = EDGES[e]
                    plus = (tt - 1) % 2 == 0
                    wslot = slot[(dy, dx)] if plus else slot[(-dy, -dx)]
                for j in range(MM_PER_G):
                    if tt == 0:
                        rhs = xpv[:, 1 + y0 + j * MM_ROWS : 1 + y0 + (j + 1) * MM_ROWS, 1 : 1 + W]
                        ws = 0
                    elif plus:
                        rhs = zps[e][:, j * MM_ROWS : (j + 1) * MM_ROWS, :]
                        ws = wslot
                    else:
                        c0 = 2 - dx
                        rhs = phis[e][:, j * MM_ROWS : j * MM_ROWS + MM_ROWS, c0 : c0 + W]
                        ws = wslot
                    nc.tensor.matmul(
                        psums[j],
                        wt[:, ws * 128 : (ws + 1) * 128],
                        rhs,
                        start=(tt == 0),
                        stop=(tt == 8),
                        skip_group_check=True,
                    )

            if g < N_GROUPS - 1:
                for j in range(MM_PER_G // 2):
                    ost = op_.tile([128, 2 * MM_ROWS * W], dt.float16, tag="ost")
                    nc.scalar.activation(
                        ost[:],
                        pstiles[j][:],
                        mybir.ActivationFunctionType.Identity,
                        bias=biast[:, 0:1],
                    )
                    c0 = (y0 + 2 * j * MM_ROWS) * W
                    q = nc.sync if g >= 6 else nc.scalar
                    q.dma_start(out_d[:, c0 : c0 + 2 * MM_ROWS * W], ost[:])
            else:
                # last chunk: quarter-granularity so the tail DMA overlaps
                for j in range(MM_PER_G):
                    ost = op_.tile([128, MM_ROWS * W], dt.float16, tag="ost4")
                    nc.scalar.activation(
                        ost[:],
                        psums[j],
                        mybir.ActivationFunctionType.Identity,
                        bias=biast[:, 0:1],
                    )
                    c0 = (y0 + j * MM_ROWS) * W
                    nc.sync.dma_start(out_d[:, c0 : c0 + MM_ROWS * W], ost[:])

    nc.compile()
    return nc


def _get_nc():
    if "nc" not in _cache:
        _cache["nc"] = _build_bass()
    return _cache["nc"]


def _host_prep(input, depth, weight, bias):
    bf16 = ml_dtypes.bfloat16

    xpad = np.zeros((B, C, HP, WP), dtype=bf16)
    xpad[:, :, 1 : 1 + H, 1 : 1 + W] = input.astype(bf16)
    xpad = xpad.reshape(B, C, HP * WP)

    # edge similarity fields on a 132x132 grid: EF[b, qy+2, qx+2] =
    # exp(-a*|D(qy,qx) - D(qy+dy,qx+dx)|), D zero-padded.
    dext = np.zeros((B, H + 6, W + 6), dtype=np.float32)
    dext[:, 3 : 3 + H, 3 : 3 + W] = depth[:, 0, :, :]
    simf = np.empty((B, N_GROUPS, 4, ER, EW), dtype=bf16)
    for e, (dy, dx) in enumerate(EDGES):
        a = dext[:, 1 : 1 + 132, 1 : 1 + 132]
        bsh = dext[:, 1 + dy : 1 + dy + 132, 1 + dx : 1 + dx + 132]
        ef = np.exp(-ALPHA * np.abs(a - bsh)).astype(bf16)  # [B, 132, 132]
        if dx == 1:
            ef[:, :, 1] = 0  # q_x=-1 zero-pad column, written by the product
        for g in range(N_GROUPS):
            simf[:, g, e, :, :] = ef[:, 16 * g + 1 : 16 * g + 1 + ER, 0:EW]
    # edges 0,1 pre-broadcast across the 128 partitions (shipped via DMA);
    # edges 2,3 stay flat (device Pool broadcast)
    simfb = np.ascontiguousarray(
        np.broadcast_to(
            simf[:, None, :, 0:2].reshape(B, 1, N_GROUPS * 2 * EBLK),
            (B, 128, N_GROUPS * 2 * EBLK),
        )
    )
    simf = np.ascontiguousarray(simf[:, :, 2:4].reshape(B, N_GROUPS, 2 * EBLK))

    wt = np.empty((C, 9 * 128), dtype=bf16)
    wtr = weight.astype(np.float32).transpose(1, 2, 3, 0)  # [c, kh, kw, o]
    wt[:, 0:128] = wtr[:, 1, 1, :].astype(bf16)
    s = 1
    for dy, dx in EDGES:
        wt[:, s * 128 : (s + 1) * 128] = wtr[:, dy + 1, dx + 1, :].astype(bf16)
        wt[:, (s + 1) * 128 : (s + 2) * 128] = wtr[:, 1 - dy, 1 - dx, :].astype(bf16)
        s += 2

    bias2 = np.ascontiguousarray(bias.astype(np.float32).reshape(128, 1))
    return xpad, simf, simfb, wt, bias2


def kernel(input, depth, weight, bias):
    from concourse.bass_utils import run_bass_kernel_spmd

    nc = _get_nc()
    xpad, simf, simfb, wt, bias2 = _host_prep(input, depth, weight, bias)

    in_maps = []
    for b in range(B):
        in_maps.append(
            {
                "xpad": np.ascontiguousarray(xpad[b]),
                "simf": np.ascontiguousarray(simf[b]),
                "simfb": simfb[b],
                "wt": wt,
                "bias": bias2,
            }
        )

    trace = os.environ.get("KERNEL_TRACE", "0") == "1"
    res = run_bass_kernel_spmd(
        nc, in_maps, core_ids=list(range(N_CORES)), trace=trace
    )
    if trace:
        _cache["last_results"] = res

    out = np.stack(
        [res.results[b]["out"].astype(np.float32).reshape(128, H, W) for b in range(B)]
    )
    return out
